# revision 15
# baseline (speedup 1.0000x reference)
"""Trainium2 Bass kernel for nn_DilatedMask: 33x33 binary mask dilation.

Computes, for x of shape (8, 2048, 2048, 1) float32 with values {0.0, 1.0}:
    mask = (x == 0)
    y    = sliding-window max of mask over a 33x33 window (SAME padding),
           as uint8.

Strategy (per NeuronCore, pure data parallel over the batch of 8):
  A square max window over a binary mask equals (2D box-sum of mask) > 0,
  and the box sum is separable: both 1-D 33-wide box sums run on the
  TensorEngine as banded matmuls with the image tile as the *stationary*
  operand, which transposes each pass's output: pass 1 (H-axis) emits a
  transposed intermediate [w, h]; pass 2 (W-axis) lands back in natural
  [h, w] orientation -- no explicit transposes.

  Mask-free pass 1: IEEE f32 {0.0, 1.0} truncates to bf16 {0.0, 1.0} in
  its top two bytes, so the raw f32 input strip, DMA'd as a bf16 tensor
  of twice the width, IS a valid {0,1} operand at stride 2 (measured:
  stride-2 bf16 LDWEIGHTS runs at contiguous-fp8 speed).  With a NEGATED
  band, pass 1 accumulates -(# ones in the H-window); SAME-padding edges
  are fixed by tiny K=16 matmuls that add -(# virtual out-of-bounds rows)
  via an all-ones stationary, so psA == n0 - 33 exactly, where n0 is the
  count of zeros (mask hits) in the in-bounds window:

    S1^T  = -33 + (zeros in H-window), out [w, h]   (PE, bf16 matmuls)
    S1c   = S1^T recentred to counts >= 0, fp8      (DVE is_gt -32.5 {0,1}
                                                     / ACT Copy bias 33)
    S2    = banded sum over W of S1c, out [h, w]    (PE, fp8 matmuls)
    y     = (S2 > 0) as uint8                       (DVE/ACT, PSUM -> SBUF)

  All values are exact at every step (integer counts; fp8 rounding of
  counts 17..33 preserves nonzero-ness), so rel err is 0.

Engine budget per core (HBM floor ~52us: 16MiB f32 in + 4MiB u8 out at
the ~400GB/s measured aggregate DMA rate):
  - no mask ops: DVE/ACT only drain/evacuate PSUM (8.4M elems at the
    ~107/118 G elem/s PSUM-port rates = ~37us combined), below the DMA
    floor, so input strips stream without elementwise back-pressure.
  - all 16 input strips stay resident in SBUF (16 x 1MiB bf16 view), so
    input DMA issue never blocks on buffer recycling.
  - output stores issue from the otherwise-idle GpSimd queue so the Sync
    queue carries only the input stream.
  - H processed in 6 progressive groups (128,512,512,512,256,128):
    narrow first group so the PE starts after two strips, narrow last
    group to shorten the tail after the final strip arrives.  Pass 2 of
    group g-1 interleaves into pass 1 of group g so the PE always has
    independent work while DVE/ACT drain the other pass's PSUM tiles.
"""

from contextlib import ExitStack

import numpy as np
import ml_dtypes

RADIUS = 16
SE = 2 * RADIUS + 1  # 33
P = 128
BANDW = P + 2 * RADIUS  # 160: out-columns reachable from one 128-row k-tile
BANK = 512  # PSUM bank width in f32 elements
H = W = 2048
N_CORES = 8

# Progressive H output groups: narrow first (early PE start), narrow last
# (short tail after the final input strip).  Each group's width is <= 512
# so it occupies exactly one PSUM bank per w-strip.
GROUPS = [(0, 128), (128, 640), (640, 1152), (1152, 1664), (1664, 1920),
          (1920, 2048)]

# Filler matmuls per w-strip per group (PE clock-warming, see below).
FILLERS = {1: 2, 2: 2, 3: 1}

# aux16 layout (bf16 [128, 320]): [0:160) negated pass-1 band;
# [160:288) rows 0..15 all-ones stationary for the edge matmuls;
# [288:304) top-edge moving; [304:320) bottom-edge moving.
A_ONES = 160
A_TOP = 288
A_BOT = 304


def band_np() -> np.ndarray:
    """Band matrix chunk [128, 160]: band[p, j] = 1 iff j-32 <= p <= j."""
    p = np.arange(P)[:, None]
    j = np.arange(BANDW)[None, :]
    return ((p <= j) & (p >= j - 2 * RADIUS)).astype(np.float32)


def aux_np() -> np.ndarray:
    """Pass-1 aux constants, bf16 [128, 320] (see A_* layout above)."""
    aux = np.zeros((P, 320), np.float32)
    aux[:, :BANDW] = -band_np()
    aux[:16, A_ONES : A_ONES + P] = 1.0
    k = np.arange(16)[:, None]
    c = np.arange(16)[None, :]
    # virtual rows above the image: row -16+k covers out-cols c <= k
    aux[:16, A_TOP : A_TOP + 16] = np.where(c <= k, -1.0, 0.0)
    # virtual rows below: row h+k covers the last 16 out-cols cb >= k
    aux[:16, A_BOT : A_BOT + 16] = np.where(c >= k, -1.0, 0.0)
    return aux.astype(ml_dtypes.bfloat16)


def _split_at(lo: int, hi: int, cuts):
    out = []
    for c in cuts:
        if lo < c < hi:
            out.append((lo, c))
            lo = c
    out.append((lo, hi))
    return out


def _pieces_for_pass(n: int, cuts):
    """Matmul pieces for one banded-sum pass with n output columns.

    One merged 160-wide output window per 128-row k-tile, split at the
    given cuts; overlap with the neighbouring k-tile's window accumulates
    via the PSUM has_written bits.  Returns list of (kt, lo, hi).
    """
    cuts = sorted(set(cuts))
    nt = n // P
    raw = []
    for kt in range(nt):
        win_lo = max(0, P * kt - RADIUS)
        win_hi = min(n, P * kt + P + RADIUS)
        for lo, hi in _split_at(win_lo, win_hi, cuts):
            raw.append((kt, lo, hi))
    return raw


def _flag_pieces(raw, tile_base):
    """Assign PSUM start/stop flags for one destination tile's piece list.

    Bank key is relative to the tile base (the tile is bank-aligned); per
    bank the first piece gets start=True, the last stop=True.
    """
    first_in_bank = {}
    last_in_bank = {}
    for i, (kt, lo, hi) in enumerate(raw):
        b = (lo - tile_base) // BANK
        if b not in first_in_bank:
            first_in_bank[b] = i
        last_in_bank[b] = i
    return [
        (kt, lo, hi,
         i == first_in_bank[(lo - tile_base) // BANK],
         i == last_in_bank[(lo - tile_base) // BANK])
        for i, (kt, lo, hi) in enumerate(raw)
    ]


def _dedupe_ldweights(nc):
    """Remove back-to-back duplicate LDWEIGHTS in the PE stream.

    Tile lowers every matmul to LDWEIGHTS+MATMUL; consecutive matmuls that
    share a stationary (the fresh/accumulate piece pairs here) reload the
    identical weights.  The PE pairs each MATMUL with the most recent
    preceding LDWEIGHTS, so the reload is dead -- and LDWEIGHTS streaming
    is a real cost in the PE stream.  Only drops loads with empty sync_info.
    """
    import concourse.mybir as mybir

    for blk in nc.m.functions[0].blocks:
        insts = list(blk.instructions)
        keep = []
        remap = {}
        last_sig = None
        for i in insts:
            if i.engine == mybir.EngineType.PE:
                tn = type(i).__name__
                if tn == "InstLdweights":
                    ap = i.ins[0]
                    sig = (ap.memref, ap.offset, str(ap.ap), str(ap.dtype),
                           bool(i.is_transpose), str(i.perf_mode))
                    si = i.sync_info
                    clean = si is None or (
                        not si.on_wait and not si.on_update
                    )
                    if sig == last_sig and clean:
                        remap[i.name] = last_name
                        continue
                    last_sig = sig
                    last_name = i.name
                elif tn != "InstMatmult":
                    last_sig = None
            keep.append(i)
        if remap:
            for i in keep:
                i.remap_dependency_names(remap)
            blk.instructions = keep


def build_program(h: int = H, w: int = W):
    """Build the per-core Bass program (SPMD, identical on all cores)."""
    import concourse.mybir as mybir
    import concourse.tile as tile
    from concourse import bacc

    f32 = mybir.dt.float32
    bf16 = mybir.dt.bfloat16
    fp8 = mybir.dt.float8e4
    u8 = mybir.dt.uint8

    nt_h = h // P
    nt_w = w // P
    groups = GROUPS if h == H else [(a, min(b, h)) for a, b in GROUPS if a < h]
    n_q = max(1, w // BANK)
    q_w = w // n_q

    nc = bacc.Bacc("TRN2", target_bir_lowering=False, debug=False)
    # x is the raw f32 image reinterpreted as bf16 [h, 2w]: the odd
    # columns are the f32 values' top halves, exactly bf16 {0.0, 1.0}.
    x_ap = nc.dram_tensor("x", [h, 2 * w], bf16, kind="ExternalInput").ap()
    band8_ap = nc.dram_tensor("band8", [P, BANDW], fp8, kind="ExternalInput").ap()
    aux_ap = nc.dram_tensor("aux16", [P, 320], bf16, kind="ExternalInput").ap()
    y_ap = nc.dram_tensor("y", [h, w], u8, kind="ExternalOutput").ap()

    group_edges = sorted({a for a, _ in groups} | {b for _, b in groups})
    pieces_h = _pieces_for_pass(h, cuts=group_edges)
    pieces_w = _pieces_for_pass(w, cuts=range(BANK, w, BANK))
    ph_by_group = {
        gi: _flag_pieces(
            [p for p in pieces_h if glo <= p[1] < ghi], tile_base=glo
        )
        for gi, (glo, ghi) in enumerate(groups)
    }
    pw_by_q = {
        qf: _flag_pieces(
            [p for p in pieces_w if qf * q_w <= p[1] < (qf + 1) * q_w],
            tile_base=qf * q_w,
        )
        for qf in range(n_q)
    }

    OGRP = 2  # output row-strips per store DMA
    n_ogrp = max(1, nt_h // OGRP)
    ogrp = nt_h // n_ogrp

    with tile.TileContext(nc) as tc, ExitStack() as ctx:
        band_pool = ctx.enter_context(tc.tile_pool(name="band", bufs=1))
        xf_pool = ctx.enter_context(tc.tile_pool(name="xf", bufs=nt_h))
        s1_pool = ctx.enter_context(tc.tile_pool(name="s1", bufs=1))
        psA_pool = ctx.enter_context(tc.tile_pool(name="psA", bufs=4, space="PSUM"))
        psF_pool = ctx.enter_context(tc.tile_pool(name="psF", bufs=1, space="PSUM"))
        psB_pool = ctx.enter_context(tc.tile_pool(name="psB", bufs=3, space="PSUM"))
        out_pool = ctx.enter_context(tc.tile_pool(name="out", bufs=4))

        band8_t = band_pool.tile([P, BANDW], fp8, tag="band8")
        nc.sync.dma_start(out=band8_t[:], in_=band8_ap[:, :])
        aux_t = band_pool.tile([P, 320], bf16, tag="aux16")
        nc.sync.dma_start(out=aux_t[:], in_=aux_ap[:, :])
        bandn_t = aux_t[:, :BANDW]
        ones_stat = aux_t[0:16, A_ONES : A_ONES + P]
        top_mov = aux_t[0:16, A_TOP : A_TOP + 16]
        bot_mov = aux_t[0:16, A_BOT : A_BOT + 16]

        # Persistent PSUM tiles (same memref -> PE WAW stays program-order).
        # psA pair-tiles hold two adjacent w-strips' pass-1 windows side by
        # side (one 512-col bank each).
        psA_tiles = [
            psA_pool.tile([P, BANK], f32, tag="psA", name=f"psA{i}")
            for i in range(4)
        ]
        psB_tiles = [
            psB_pool.tile([P, BANK], f32, tag="psB", name=f"psB{i}")
            for i in range(3)
        ]
        psF = psF_pool.tile([P, BANK], f32, tag="psF")
        nB = [0]

        def next_psB():
            t = psB_tiles[nB[0] % len(psB_tiles)]
            nB[0] += 1
            return t

        # PE warm-up: throwaway matmuls on the band tile while the first
        # input strips stream in.  Sustained PE activity lifts the HAM
        # clock gate (1.2 -> 2.4 GHz) before the real work arrives.  psB0
        # is clobbered; its first real matmul starts with start=True.
        for _ in range(64):
            nc.tensor.matmul(
                psB_tiles[0][:, :64],
                band8_t[:, :P],
                band8_t[:, :64],
                start=True,
                stop=True,
            )

        # Input: contiguous row strips of the raw f32 bytes as bf16
        # [128, 2w]; consumed directly by pass-1 LDWEIGHTS at stride 2.
        # All strips stay resident, so this stream never blocks.
        xf_by_kt = {}
        for kt in range(nt_h):
            xf = xf_pool.tile([P, 2 * w], bf16, tag="xf")
            nc.sync.dma_start(
                out=xf[:], in_=x_ap[kt * P : (kt + 1) * P, :]
            )
            xf_by_kt[kt] = xf

        # S1^T strips: one [128, nt_w*h] fp8 tensor (counts 0..33, fp8
        # rounding preserves zero vs nonzero), column block wt holds
        # S1^T[wt] = [w', h].
        s1_t = s1_pool.tile([P, nt_w * h], fp8, tag="s1")
        s13 = s1_t[:].rearrange("p (wt ht) -> p wt ht", wt=nt_w)

        yt_tiles = {}
        done_ht = {}

        # Drains and evacs each alternate deterministically across DVE
        # and ACT (indexed, not flip-shared), so both engines carry half
        # of each stream and each queue's op order matches the order the
        # PE produces their PSUM inputs (no head-of-line blocking).
        # drain: psA (= n0 - 33) -> s1; DVE binarizes via is_gt -32.5,
        #   ACT recentres to raw counts via Copy bias 33 (mixing {0,1}
        #   and count blocks is fine: pass 2 needs zero vs nonzero).
        # evac: psB (>= 0) -> final {0,1} uint8.
        def drain_op(idx, dst_ap, src_ap):
            if idx % 2 == 0:
                nc.vector.tensor_scalar(
                    dst_ap, src_ap, -32.5, None, mybir.AluOpType.is_gt
                )
            else:
                nc.scalar.activation(
                    dst_ap, src_ap, mybir.ActivationFunctionType.Copy,
                    bias=33.0, scale=1.0,
                )

        def evac_bin(idx, dst_ap, src_ap):
            if idx % 2 == 0:
                nc.scalar.sign(dst_ap, src_ap)
            else:
                nc.vector.tensor_scalar(
                    dst_ap, src_ap, 0.5, None, mybir.AluOpType.is_gt
                )

        def p2_unit(ht, qf):
            """One pass-2 unit: fill a psB quarter for row-strip ht, evac
            it, and store the row-group when complete."""
            og, a = divmod(ht, ogrp)
            if og not in yt_tiles:
                yt_tiles[og] = out_pool.tile(
                    [P, ogrp * w], u8, tag="yt", name=f"yt{og}"
                )
                done_ht[og] = 0
            yt = yt_tiles[og]
            psB = next_psB()
            for wt, lo, hi, st, sp in pw_by_q[qf]:
                base = P * wt - RADIUS
                nc.tensor.matmul(
                    psB[:, lo - qf * q_w : hi - qf * q_w],
                    s1_t[:, wt * h + ht * P : wt * h + (ht + 1) * P],
                    band8_t[:, lo - base : hi - base],
                    start=st,
                    stop=sp,
                )
            if og == n_ogrp - 1:
                # Terminal chain: halve the evacuation latency by running
                # the two halves on DVE and ACT in parallel, and store the
                # final rows per strip so the very last store is 256KB.
                base = a * w + qf * q_w
                mid = q_w // 2
                nc.vector.tensor_scalar(
                    yt[:, base : base + mid], psB[:, :mid],
                    0.5, None, mybir.AluOpType.is_gt,
                )
                nc.scalar.sign(
                    yt[:, base + mid : base + q_w], psB[:, mid:q_w]
                )
                if qf == n_q - 1:
                    nc.sync.dma_start(
                        out=y_ap[ht * P : (ht + 1) * P, :],
                        in_=yt[:, a * w : (a + 1) * w],
                    )
                done_ht[og] += 1
                return
            evac_bin(
                ht * n_q + qf,
                yt[:, a * w + qf * q_w : a * w + (qf + 1) * q_w],
                psB[:, :q_w],
            )
            done_ht[og] += 1
            if done_ht[og] == ogrp * n_q:
                dst = y_ap[og * ogrp * P : (og + 1) * ogrp * P, :].rearrange(
                    "(a p) w -> p a w", p=P
                )
                nc.sync.dma_start(
                    out=dst, in_=yt[:].rearrange("p (a w) -> p a w", a=ogrp)
                )

        # pending pass-2 units from the previous group, interleaved into
        # this group's pass 1 so the PE always has independent work while
        # DVE/ACT drain the other pass's PSUM tiles.
        pending_p2 = []

        for gi, (glo, ghi) in enumerate(groups):
            gw = ghi - glo
            edge = "top" if gi == 0 else ("bot" if gi == len(groups) - 1
                                          else None)
            # Pass 1 for this group: psum cols are h_out in [glo, ghi);
            # w-strip pairs share one psA tile (bank 0 / bank 1).
            for wt in range(nt_w):
                psA = psA_tiles[wt % 4]
                side = 0
                if edge == "top":
                    nc.tensor.matmul(
                        psA[:, side : side + 16],
                        ones_stat, top_mov, start=True, stop=False,
                    )
                elif edge == "bot":
                    nc.tensor.matmul(
                        psA[:, side + gw - 16 : side + gw],
                        ones_stat, bot_mov, start=True, stop=False,
                    )
                for kt, lo, hi, st, sp in ph_by_group[gi]:
                    base = P * kt - RADIUS
                    xv = xf_by_kt[kt][:].rearrange(
                        "p (c two) -> p c two", two=2
                    )
                    nc.tensor.matmul(
                        psA[:, side + lo - glo : side + hi - glo],
                        xv[:, wt * P : (wt + 1) * P, 1:2],
                        bandn_t[:, lo - base : hi - base],
                        start=st and edge is None,
                        stop=sp,
                    )
                for _ in range(FILLERS.get(gi, 0)):
                    # Filler matmul on the dedicated bank: keeps the PE
                    # HAM clock warm through the DMA-paced early groups
                    # (an idle PE re-throttles to half clock).  No data
                    # deps, result discarded.
                    nc.tensor.matmul(
                        psF[:, :BANK], band8_t[:, :P],
                        xf_by_kt[0][:, :BANK], start=True, stop=True,
                    )
                if gi == len(groups) - 1:
                    # terminal chain: split each drain across DVE+ACT in
                    # parallel to halve its latency
                    mid = gw // 2
                    nc.vector.tensor_scalar(
                        s13[:, wt : wt + 1, glo : glo + mid],
                        psA[:, :mid],
                        -32.5, None, mybir.AluOpType.is_gt,
                    )
                    nc.scalar.activation(
                        s13[:, wt : wt + 1, glo + mid : ghi],
                        psA[:, mid:gw],
                        mybir.ActivationFunctionType.Copy,
                        bias=33.0, scale=1.0,
                    )
                else:
                    drain_op(wt, s13[:, wt : wt + 1, glo:ghi], psA[:, :gw])
                if pending_p2:
                    p2_unit(*pending_p2.pop(0))
            pending_p2.extend(
                (ht, qf)
                for ht in range(glo // P, ghi // P)
                for qf in range(n_q)
            )
            if gi < len(groups) - 1:
                # Flush this group's pass-2 eagerly, BEFORE the next
                # group's pass-1 enters the in-order PE queue: the next
                # group's matmuls wait on input strips, and ready pass-2
                # units emitted behind them would head-of-line block.
                while pending_p2:
                    p2_unit(*pending_p2.pop(0))

        for u in pending_p2:
            p2_unit(*u)

    _dedupe_ldweights(nc)
    nc.compile()
    return nc


def make_in_maps(imgs: np.ndarray):
    """Per-core input dicts from the [N, h, w] f32 image stack."""
    xbf = imgs.view(ml_dtypes.bfloat16)  # [N, h, 2w], same bytes
    band8 = band_np().astype(ml_dtypes.float8_e4m3)
    aux = aux_np()
    return [
        {"x": xbf[c], "band8": band8, "aux16": aux}
        for c in range(imgs.shape[0])
    ]


def kernel(x: np.ndarray) -> np.ndarray:
    """Full-input entry point: x (8, 2048, 2048, 1) f32 -> y same shape uint8."""
    from concourse.bass_utils import run_bass_kernel_spmd

    x = np.asarray(x)
    assert x.shape == (N_CORES, H, W, 1), x.shape
    imgs = np.ascontiguousarray(x[:, :, :, 0], dtype=np.float32)

    nc = build_program(H, W)
    res = run_bass_kernel_spmd(nc, make_in_maps(imgs), list(range(N_CORES)))
    y = np.stack([res.results[c]["y"] for c in range(N_CORES)])
    return y[..., None]


# revision 16
# speedup vs baseline: 1.0516x; 1.0516x over previous
"""Trainium2 Bass kernel for nn_DilatedMask: 33x33 binary mask dilation.

Computes, for x of shape (8, 2048, 2048, 1) float32 with values {0.0, 1.0}:
    mask = (x == 0)
    y    = sliding-window max of mask over a 33x33 window (SAME padding),
           as uint8.

Strategy (per NeuronCore, pure data parallel over the batch of 8):
  A square max window over a binary mask equals (2D box-sum of mask) > 0,
  and the box sum is separable: both 1-D 33-wide box sums run on the
  TensorEngine as banded matmuls with the image tile as the *stationary*
  operand, which transposes each pass's output: pass 1 (H-axis) emits a
  transposed intermediate [w, h]; pass 2 (W-axis) lands back in natural
  [h, w] orientation -- no explicit transposes.

  Mask-free pass 1: IEEE f32 {0.0, 1.0} truncates to bf16 {0.0, 1.0} in
  its top two bytes, so the raw f32 input strip, DMA'd as a bf16 tensor
  of twice the width, IS a valid {0,1} operand at stride 2 (measured:
  stride-2 bf16 LDWEIGHTS runs at contiguous-fp8 speed).  With a NEGATED
  band, pass 1 accumulates -(# ones in the H-window); SAME-padding edges
  are fixed by tiny K=16 matmuls that add -(# virtual out-of-bounds rows)
  via an all-ones stationary, so psA == n0 - 33 exactly, where n0 is the
  count of zeros (mask hits) in the in-bounds window:

    S1^T  = -33 + (zeros in H-window), out [w, h]   (PE, bf16 matmuls)
    S1c   = S1^T recentred to counts >= 0, fp8      (DVE is_gt -32.5 {0,1}
                                                     / ACT Copy bias 33)
    S2    = banded sum over W of S1c, out [h, w]    (PE, fp8 matmuls)
    y     = (S2 > 0) as uint8                       (DVE/ACT, PSUM -> SBUF)

  All values are exact at every step (integer counts; fp8 rounding of
  counts 17..33 preserves nonzero-ness), so rel err is 0.

Engine budget per core (HBM floor ~52us: 16MiB f32 in + 4MiB u8 out at
the ~400GB/s measured aggregate DMA rate):
  - no mask ops: DVE/ACT only drain/evacuate PSUM (8.4M elems at the
    ~107/118 G elem/s PSUM-port rates = ~37us combined), below the DMA
    floor, so input strips stream without elementwise back-pressure.
  - all 16 input strips stay resident in SBUF (16 x 1MiB bf16 view), so
    input DMA issue never blocks on buffer recycling.
  - output stores issue from the otherwise-idle GpSimd queue so the Sync
    queue carries only the input stream.
  - H processed in 6 progressive groups (128,512,512,512,256,128):
    narrow first group so the PE starts after two strips, narrow last
    group to shorten the tail after the final strip arrives.  Pass 2 of
    group g-1 interleaves into pass 1 of group g so the PE always has
    independent work while DVE/ACT drain the other pass's PSUM tiles.
"""

from contextlib import ExitStack

import numpy as np
import ml_dtypes

RADIUS = 16
SE = 2 * RADIUS + 1  # 33
P = 128
BANDW = P + 2 * RADIUS  # 160: out-columns reachable from one 128-row k-tile
BANK = 512  # PSUM bank width in f32 elements
H = W = 2048
N_CORES = 8

# Progressive H output groups: narrow first (early PE start), narrow last
# (short tail after the final input strip).  Each group's width is <= 512
# so it occupies exactly one PSUM bank per w-strip.
GROUPS = [(0, 128), (128, 640), (640, 1152), (1152, 1664), (1664, 1920),
          (1920, 2048)]

# Filler matmuls per w-strip per group (PE clock-warming, see below).
FILLERS = {1: 1, 2: 1, 3: 1}

# aux16 layout (bf16 [128, 320]): [0:160) negated pass-1 band;
# [160:288) rows 0..15 all-ones stationary for the edge matmuls;
# [288:304) top-edge moving; [304:320) bottom-edge moving.
A_ONES = 160
A_TOP = 288
A_BOT = 304


def band_np() -> np.ndarray:
    """Band matrix chunk [128, 160]: band[p, j] = 1 iff j-32 <= p <= j."""
    p = np.arange(P)[:, None]
    j = np.arange(BANDW)[None, :]
    return ((p <= j) & (p >= j - 2 * RADIUS)).astype(np.float32)


def aux_np() -> np.ndarray:
    """Pass-1 aux constants, bf16 [128, 320] (see A_* layout above)."""
    aux = np.zeros((P, 320), np.float32)
    aux[:, :BANDW] = -band_np()
    aux[:16, A_ONES : A_ONES + P] = 1.0
    k = np.arange(16)[:, None]
    c = np.arange(16)[None, :]
    # virtual rows above the image: row -16+k covers out-cols c <= k
    aux[:16, A_TOP : A_TOP + 16] = np.where(c <= k, -1.0, 0.0)
    # virtual rows below: row h+k covers the last 16 out-cols cb >= k
    aux[:16, A_BOT : A_BOT + 16] = np.where(c >= k, -1.0, 0.0)
    return aux.astype(ml_dtypes.bfloat16)


def _split_at(lo: int, hi: int, cuts):
    out = []
    for c in cuts:
        if lo < c < hi:
            out.append((lo, c))
            lo = c
    out.append((lo, hi))
    return out


def _pieces_for_pass(n: int, cuts):
    """Matmul pieces for one banded-sum pass with n output columns.

    One merged 160-wide output window per 128-row k-tile, split at the
    given cuts; overlap with the neighbouring k-tile's window accumulates
    via the PSUM has_written bits.  Returns list of (kt, lo, hi).
    """
    cuts = sorted(set(cuts))
    nt = n // P
    raw = []
    for kt in range(nt):
        win_lo = max(0, P * kt - RADIUS)
        win_hi = min(n, P * kt + P + RADIUS)
        for lo, hi in _split_at(win_lo, win_hi, cuts):
            raw.append((kt, lo, hi))
    return raw


def _flag_pieces(raw, tile_base):
    """Assign PSUM start/stop flags for one destination tile's piece list.

    Bank key is relative to the tile base (the tile is bank-aligned); per
    bank the first piece gets start=True, the last stop=True.
    """
    first_in_bank = {}
    last_in_bank = {}
    for i, (kt, lo, hi) in enumerate(raw):
        b = (lo - tile_base) // BANK
        if b not in first_in_bank:
            first_in_bank[b] = i
        last_in_bank[b] = i
    return [
        (kt, lo, hi,
         i == first_in_bank[(lo - tile_base) // BANK],
         i == last_in_bank[(lo - tile_base) // BANK])
        for i, (kt, lo, hi) in enumerate(raw)
    ]


def _dedupe_ldweights(nc):
    """Remove back-to-back duplicate LDWEIGHTS in the PE stream.

    Tile lowers every matmul to LDWEIGHTS+MATMUL; consecutive matmuls that
    share a stationary (the fresh/accumulate piece pairs here) reload the
    identical weights.  The PE pairs each MATMUL with the most recent
    preceding LDWEIGHTS, so the reload is dead -- and LDWEIGHTS streaming
    is a real cost in the PE stream.  Only drops loads with empty sync_info.
    """
    import concourse.mybir as mybir

    for blk in nc.m.functions[0].blocks:
        insts = list(blk.instructions)
        keep = []
        remap = {}
        last_sig = None
        for i in insts:
            if i.engine == mybir.EngineType.PE:
                tn = type(i).__name__
                if tn == "InstLdweights":
                    ap = i.ins[0]
                    sig = (ap.memref, ap.offset, str(ap.ap), str(ap.dtype),
                           bool(i.is_transpose), str(i.perf_mode))
                    si = i.sync_info
                    clean = si is None or (
                        not si.on_wait and not si.on_update
                    )
                    if sig == last_sig and clean:
                        remap[i.name] = last_name
                        continue
                    last_sig = sig
                    last_name = i.name
                elif tn != "InstMatmult":
                    last_sig = None
            keep.append(i)
        if remap:
            for i in keep:
                i.remap_dependency_names(remap)
            blk.instructions = keep


def build_program(h: int = H, w: int = W):
    """Build the per-core Bass program (SPMD, identical on all cores)."""
    import concourse.mybir as mybir
    import concourse.tile as tile
    from concourse import bacc

    f32 = mybir.dt.float32
    bf16 = mybir.dt.bfloat16
    fp8 = mybir.dt.float8e4
    u8 = mybir.dt.uint8

    nt_h = h // P
    nt_w = w // P
    groups = GROUPS if h == H else [(a, min(b, h)) for a, b in GROUPS if a < h]
    n_q = max(1, w // BANK)
    q_w = w // n_q

    nc = bacc.Bacc("TRN2", target_bir_lowering=False, debug=False)
    # x is the raw f32 image reinterpreted as bf16 [h, 2w]: the odd
    # columns are the f32 values' top halves, exactly bf16 {0.0, 1.0}.
    x_ap = nc.dram_tensor("x", [h, 2 * w], bf16, kind="ExternalInput").ap()
    band8_ap = nc.dram_tensor("band8", [P, BANDW], fp8, kind="ExternalInput").ap()
    aux_ap = nc.dram_tensor("aux16", [P, 320], bf16, kind="ExternalInput").ap()
    y_ap = nc.dram_tensor("y", [h, w], u8, kind="ExternalOutput").ap()

    group_edges = sorted({a for a, _ in groups} | {b for _, b in groups})
    pieces_h = _pieces_for_pass(h, cuts=group_edges)
    pieces_w = _pieces_for_pass(w, cuts=range(BANK, w, BANK))
    ph_by_group = {
        gi: _flag_pieces(
            [p for p in pieces_h if glo <= p[1] < ghi], tile_base=glo
        )
        for gi, (glo, ghi) in enumerate(groups)
    }
    pw_by_q = {
        qf: _flag_pieces(
            [p for p in pieces_w if qf * q_w <= p[1] < (qf + 1) * q_w],
            tile_base=qf * q_w,
        )
        for qf in range(n_q)
    }

    OGRP = 2  # output row-strips per store DMA
    n_ogrp = max(1, nt_h // OGRP)
    ogrp = nt_h // n_ogrp

    with tile.TileContext(nc) as tc, ExitStack() as ctx:
        band_pool = ctx.enter_context(tc.tile_pool(name="band", bufs=1))
        xf_pool = ctx.enter_context(tc.tile_pool(name="xf", bufs=nt_h))
        s1_pool = ctx.enter_context(tc.tile_pool(name="s1", bufs=1))
        psA_pool = ctx.enter_context(tc.tile_pool(name="psA", bufs=4, space="PSUM"))
        psF_pool = ctx.enter_context(tc.tile_pool(name="psF", bufs=1, space="PSUM"))
        psB_pool = ctx.enter_context(tc.tile_pool(name="psB", bufs=3, space="PSUM"))
        out_pool = ctx.enter_context(tc.tile_pool(name="out", bufs=4))

        band8_t = band_pool.tile([P, BANDW], fp8, tag="band8")
        nc.sync.dma_start(out=band8_t[:], in_=band8_ap[:, :])
        aux_t = band_pool.tile([P, 320], bf16, tag="aux16")
        nc.sync.dma_start(out=aux_t[:], in_=aux_ap[:, :])
        bandn_t = aux_t[:, :BANDW]
        ones_stat = aux_t[0:16, A_ONES : A_ONES + P]
        top_mov = aux_t[0:16, A_TOP : A_TOP + 16]
        bot_mov = aux_t[0:16, A_BOT : A_BOT + 16]

        # Persistent PSUM tiles (same memref -> PE WAW stays program-order).
        # psA pair-tiles hold two adjacent w-strips' pass-1 windows side by
        # side (one 512-col bank each).
        psA_tiles = [
            psA_pool.tile([P, BANK], f32, tag="psA", name=f"psA{i}")
            for i in range(4)
        ]
        psB_tiles = [
            psB_pool.tile([P, BANK], f32, tag="psB", name=f"psB{i}")
            for i in range(3)
        ]
        psF = psF_pool.tile([P, BANK], f32, tag="psF")
        nB = [0]

        def next_psB():
            t = psB_tiles[nB[0] % len(psB_tiles)]
            nB[0] += 1
            return t

        # PE warm-up: throwaway matmuls on the band tile while the first
        # input strips stream in.  Sustained PE activity lifts the HAM
        # clock gate (1.2 -> 2.4 GHz) before the real work arrives.  psB0
        # is clobbered; its first real matmul starts with start=True.
        for _ in range(64):
            nc.tensor.matmul(
                psB_tiles[0][:, :64],
                band8_t[:, :P],
                band8_t[:, :64],
                start=True,
                stop=True,
            )

        # Input: contiguous row strips of the raw f32 bytes as bf16
        # [128, 2w]; consumed directly by pass-1 LDWEIGHTS at stride 2.
        # All strips stay resident, so this stream never blocks.
        xf_by_kt = {}
        for kt in range(nt_h):
            xf = xf_pool.tile([P, 2 * w], bf16, tag="xf")
            nc.sync.dma_start(
                out=xf[:], in_=x_ap[kt * P : (kt + 1) * P, :]
            )
            xf_by_kt[kt] = xf

        # S1^T strips: one [128, nt_w*h] fp8 tensor (counts 0..33, fp8
        # rounding preserves zero vs nonzero), column block wt holds
        # S1^T[wt] = [w', h].
        s1_t = s1_pool.tile([P, nt_w * h], fp8, tag="s1")
        s13 = s1_t[:].rearrange("p (wt ht) -> p wt ht", wt=nt_w)

        yt_tiles = {}
        done_ht = {}

        # Drains and evacs each alternate deterministically across DVE
        # and ACT (indexed, not flip-shared), so both engines carry half
        # of each stream and each queue's op order matches the order the
        # PE produces their PSUM inputs (no head-of-line blocking).
        # drain: psA (= n0 - 33) -> s1; DVE binarizes via is_gt -32.5,
        #   ACT recentres to raw counts via Copy bias 33 (mixing {0,1}
        #   and count blocks is fine: pass 2 needs zero vs nonzero).
        # evac: psB (>= 0) -> final {0,1} uint8.
        def drain_op(idx, dst_ap, src_ap):
            if idx % 2 == 0:
                nc.vector.tensor_scalar(
                    dst_ap, src_ap, -32.5, None, mybir.AluOpType.is_gt
                )
            else:
                nc.scalar.activation(
                    dst_ap, src_ap, mybir.ActivationFunctionType.Copy,
                    bias=33.0, scale=1.0,
                )

        def evac_bin(idx, dst_ap, src_ap):
            if idx % 2 == 0:
                nc.scalar.sign(dst_ap, src_ap)
            else:
                nc.vector.tensor_scalar(
                    dst_ap, src_ap, 0.5, None, mybir.AluOpType.is_gt
                )

        def p2_unit(ht, qf):
            """One pass-2 unit: fill a psB quarter for row-strip ht, evac
            it, and store the row-group when complete."""
            og, a = divmod(ht, ogrp)
            if og not in yt_tiles:
                yt_tiles[og] = out_pool.tile(
                    [P, ogrp * w], u8, tag="yt", name=f"yt{og}"
                )
                done_ht[og] = 0
            yt = yt_tiles[og]
            psB = next_psB()
            for wt, lo, hi, st, sp in pw_by_q[qf]:
                base = P * wt - RADIUS
                nc.tensor.matmul(
                    psB[:, lo - qf * q_w : hi - qf * q_w],
                    s1_t[:, wt * h + ht * P : wt * h + (ht + 1) * P],
                    band8_t[:, lo - base : hi - base],
                    start=st,
                    stop=sp,
                )
            if og == n_ogrp - 1:
                # Terminal chain: halve the evacuation latency by running
                # the two halves on DVE and ACT in parallel, and store the
                # final rows per strip so the very last store is 256KB.
                base = a * w + qf * q_w
                mid = q_w // 2
                nc.vector.tensor_scalar(
                    yt[:, base : base + mid], psB[:, :mid],
                    0.5, None, mybir.AluOpType.is_gt,
                )
                nc.scalar.sign(
                    yt[:, base + mid : base + q_w], psB[:, mid:q_w]
                )
                if qf == n_q - 1:
                    nc.sync.dma_start(
                        out=y_ap[ht * P : (ht + 1) * P, :],
                        in_=yt[:, a * w : (a + 1) * w],
                    )
                done_ht[og] += 1
                return
            evac_bin(
                ht * n_q + qf,
                yt[:, a * w + qf * q_w : a * w + (qf + 1) * q_w],
                psB[:, :q_w],
            )
            done_ht[og] += 1
            if done_ht[og] == ogrp * n_q:
                dst = y_ap[og * ogrp * P : (og + 1) * ogrp * P, :].rearrange(
                    "(a p) w -> p a w", p=P
                )
                nc.sync.dma_start(
                    out=dst, in_=yt[:].rearrange("p (a w) -> p a w", a=ogrp)
                )

        # pending pass-2 units from the previous group, interleaved into
        # this group's pass 1 so the PE always has independent work while
        # DVE/ACT drain the other pass's PSUM tiles.
        pending_p2 = []

        for gi, (glo, ghi) in enumerate(groups):
            gw = ghi - glo
            edge = "top" if gi == 0 else ("bot" if gi == len(groups) - 1
                                          else None)
            # Pass 1 for this group: psum cols are h_out in [glo, ghi);
            # w-strip pairs share one psA tile (bank 0 / bank 1).
            for wt in range(nt_w):
                psA = psA_tiles[wt % 4]
                side = 0
                if edge == "top":
                    nc.tensor.matmul(
                        psA[:, side : side + 16],
                        ones_stat, top_mov, start=True, stop=False,
                    )
                elif edge == "bot":
                    nc.tensor.matmul(
                        psA[:, side + gw - 16 : side + gw],
                        ones_stat, bot_mov, start=True, stop=False,
                    )
                for kt, lo, hi, st, sp in ph_by_group[gi]:
                    base = P * kt - RADIUS
                    xv = xf_by_kt[kt][:].rearrange(
                        "p (c two) -> p c two", two=2
                    )
                    nc.tensor.matmul(
                        psA[:, side + lo - glo : side + hi - glo],
                        xv[:, wt * P : (wt + 1) * P, 1:2],
                        bandn_t[:, lo - base : hi - base],
                        start=st and edge is None,
                        stop=sp,
                    )
                for _ in range(FILLERS.get(gi, 0)):
                    # Filler matmul on the dedicated bank: keeps the PE
                    # HAM clock warm through the DMA-paced early groups
                    # (an idle PE re-throttles to half clock).  No data
                    # deps, result discarded.
                    nc.tensor.matmul(
                        psF[:, :BANK], band8_t[:, :P],
                        xf_by_kt[0][:, :BANK], start=True, stop=True,
                    )
                if gi == len(groups) - 1:
                    # terminal chain: split each drain across DVE+ACT in
                    # parallel to halve its latency
                    mid = gw // 2
                    nc.vector.tensor_scalar(
                        s13[:, wt : wt + 1, glo : glo + mid],
                        psA[:, :mid],
                        -32.5, None, mybir.AluOpType.is_gt,
                    )
                    nc.scalar.activation(
                        s13[:, wt : wt + 1, glo + mid : ghi],
                        psA[:, mid:gw],
                        mybir.ActivationFunctionType.Copy,
                        bias=33.0, scale=1.0,
                    )
                else:
                    drain_op(wt, s13[:, wt : wt + 1, glo:ghi], psA[:, :gw])
                if pending_p2:
                    p2_unit(*pending_p2.pop(0))
            pending_p2.extend(
                (ht, qf)
                for ht in range(glo // P, ghi // P)
                for qf in range(n_q)
            )
            if gi < len(groups) - 1:
                # Flush this group's pass-2 eagerly, BEFORE the next
                # group's pass-1 enters the in-order PE queue: the next
                # group's matmuls wait on input strips, and ready pass-2
                # units emitted behind them would head-of-line block.
                while pending_p2:
                    p2_unit(*pending_p2.pop(0))

        for u in pending_p2:
            p2_unit(*u)

    _dedupe_ldweights(nc)
    nc.compile()
    return nc


def make_in_maps(imgs: np.ndarray):
    """Per-core input dicts from the [N, h, w] f32 image stack."""
    xbf = imgs.view(ml_dtypes.bfloat16)  # [N, h, 2w], same bytes
    band8 = band_np().astype(ml_dtypes.float8_e4m3)
    aux = aux_np()
    return [
        {"x": xbf[c], "band8": band8, "aux16": aux}
        for c in range(imgs.shape[0])
    ]


def kernel(x: np.ndarray) -> np.ndarray:
    """Full-input entry point: x (8, 2048, 2048, 1) f32 -> y same shape uint8."""
    from concourse.bass_utils import run_bass_kernel_spmd

    x = np.asarray(x)
    assert x.shape == (N_CORES, H, W, 1), x.shape
    imgs = np.ascontiguousarray(x[:, :, :, 0], dtype=np.float32)

    nc = build_program(H, W)
    res = run_bass_kernel_spmd(nc, make_in_maps(imgs), list(range(N_CORES)))
    y = np.stack([res.results[c]["y"] for c in range(N_CORES)])
    return y[..., None]


# revision 17
# speedup vs baseline: 1.0702x; 1.0177x over previous
"""Trainium2 Bass kernel for nn_DilatedMask: 33x33 binary mask dilation.

Computes, for x of shape (8, 2048, 2048, 1) float32 with values {0.0, 1.0}:
    mask = (x == 0)
    y    = sliding-window max of mask over a 33x33 window (SAME padding),
           as uint8.

Strategy (per NeuronCore, pure data parallel over the batch of 8):
  A square max window over a binary mask equals (2D box-sum of mask) > 0,
  and the box sum is separable: both 1-D 33-wide box sums run on the
  TensorEngine as banded matmuls with the image tile as the *stationary*
  operand, which transposes each pass's output: pass 1 (H-axis) emits a
  transposed intermediate [w, h]; pass 2 (W-axis) lands back in natural
  [h, w] orientation -- no explicit transposes.

  Mask-free pass 1: IEEE f32 {0.0, 1.0} truncates to bf16 {0.0, 1.0} in
  its top two bytes, so the raw f32 input strip, DMA'd as a bf16 tensor
  of twice the width, IS a valid {0,1} operand at stride 2 (measured:
  stride-2 bf16 LDWEIGHTS runs at contiguous-fp8 speed).  With a NEGATED
  band, pass 1 accumulates -(# ones in the H-window); SAME-padding edges
  are fixed by tiny K=16 matmuls that add -(# virtual out-of-bounds rows)
  via an all-ones stationary, so psA == n0 - 33 exactly, where n0 is the
  count of zeros (mask hits) in the in-bounds window:

    S1^T  = -33 + (zeros in H-window), out [w, h]   (PE, bf16 matmuls)
    S1c   = S1^T recentred to counts >= 0, fp8      (DVE is_gt -32.5 {0,1}
                                                     / ACT Copy bias 33)
    S2    = banded sum over W of S1c, out [h, w]    (PE, fp8 matmuls)
    y     = (S2 > 0) as uint8                       (DVE/ACT, PSUM -> SBUF)

  All values are exact at every step (integer counts; fp8 rounding of
  counts 17..33 preserves nonzero-ness), so rel err is 0.

Engine budget per core (HBM floor ~52us: 16MiB f32 in + 4MiB u8 out at
the ~400GB/s measured aggregate DMA rate):
  - no mask ops: DVE/ACT only drain/evacuate PSUM (8.4M elems at the
    ~107/118 G elem/s PSUM-port rates = ~37us combined), below the DMA
    floor, so input strips stream without elementwise back-pressure.
  - all 16 input strips stay resident in SBUF (16 x 1MiB bf16 view), so
    input DMA issue never blocks on buffer recycling.
  - output stores issue from the otherwise-idle GpSimd queue so the Sync
    queue carries only the input stream.
  - H processed in 6 progressive groups (128,512,512,512,256,128):
    narrow first group so the PE starts after two strips, narrow last
    group to shorten the tail after the final strip arrives.  Pass 2 of
    group g-1 interleaves into pass 1 of group g so the PE always has
    independent work while DVE/ACT drain the other pass's PSUM tiles.
"""

from contextlib import ExitStack

import numpy as np
import ml_dtypes

RADIUS = 16
SE = 2 * RADIUS + 1  # 33
P = 128
BANDW = P + 2 * RADIUS  # 160: out-columns reachable from one 128-row k-tile
BANK = 512  # PSUM bank width in f32 elements
H = W = 2048
N_CORES = 8

# Progressive H output groups: narrow first (early PE start), narrow last
# (short tail after the final input strip).  Each group's width is <= 512
# so it occupies exactly one PSUM bank per w-strip.
GROUPS = [(0, 128), (128, 640), (640, 1152), (1152, 1664), (1664, 1920),
          (1920, 2048)]

# Filler matmuls per w-strip per group (PE clock-warming, see below).
FILLERS = {1: 1}

# aux16 layout (bf16 [128, 320]): [0:160) negated pass-1 band;
# [160:288) rows 0..15 all-ones stationary for the edge matmuls;
# [288:304) top-edge moving; [304:320) bottom-edge moving.
A_ONES = 160
A_TOP = 288
A_BOT = 304


def band_np() -> np.ndarray:
    """Band matrix chunk [128, 160]: band[p, j] = 1 iff j-32 <= p <= j."""
    p = np.arange(P)[:, None]
    j = np.arange(BANDW)[None, :]
    return ((p <= j) & (p >= j - 2 * RADIUS)).astype(np.float32)


def aux_np() -> np.ndarray:
    """Pass-1 aux constants, bf16 [128, 320] (see A_* layout above)."""
    aux = np.zeros((P, 320), np.float32)
    aux[:, :BANDW] = -band_np()
    aux[:16, A_ONES : A_ONES + P] = 1.0
    k = np.arange(16)[:, None]
    c = np.arange(16)[None, :]
    # virtual rows above the image: row -16+k covers out-cols c <= k
    aux[:16, A_TOP : A_TOP + 16] = np.where(c <= k, -1.0, 0.0)
    # virtual rows below: row h+k covers the last 16 out-cols cb >= k
    aux[:16, A_BOT : A_BOT + 16] = np.where(c >= k, -1.0, 0.0)
    return aux.astype(ml_dtypes.bfloat16)


def _split_at(lo: int, hi: int, cuts):
    out = []
    for c in cuts:
        if lo < c < hi:
            out.append((lo, c))
            lo = c
    out.append((lo, hi))
    return out


def _pieces_for_pass(n: int, cuts):
    """Matmul pieces for one banded-sum pass with n output columns.

    One merged 160-wide output window per 128-row k-tile, split at the
    given cuts; overlap with the neighbouring k-tile's window accumulates
    via the PSUM has_written bits.  Returns list of (kt, lo, hi).
    """
    cuts = sorted(set(cuts))
    nt = n // P
    raw = []
    for kt in range(nt):
        win_lo = max(0, P * kt - RADIUS)
        win_hi = min(n, P * kt + P + RADIUS)
        for lo, hi in _split_at(win_lo, win_hi, cuts):
            raw.append((kt, lo, hi))
    return raw


def _flag_pieces(raw, tile_base):
    """Assign PSUM start/stop flags for one destination tile's piece list.

    Bank key is relative to the tile base (the tile is bank-aligned); per
    bank the first piece gets start=True, the last stop=True.
    """
    first_in_bank = {}
    last_in_bank = {}
    for i, (kt, lo, hi) in enumerate(raw):
        b = (lo - tile_base) // BANK
        if b not in first_in_bank:
            first_in_bank[b] = i
        last_in_bank[b] = i
    return [
        (kt, lo, hi,
         i == first_in_bank[(lo - tile_base) // BANK],
         i == last_in_bank[(lo - tile_base) // BANK])
        for i, (kt, lo, hi) in enumerate(raw)
    ]


def _dedupe_ldweights(nc):
    """Remove back-to-back duplicate LDWEIGHTS in the PE stream.

    Tile lowers every matmul to LDWEIGHTS+MATMUL; consecutive matmuls that
    share a stationary (the fresh/accumulate piece pairs here) reload the
    identical weights.  The PE pairs each MATMUL with the most recent
    preceding LDWEIGHTS, so the reload is dead -- and LDWEIGHTS streaming
    is a real cost in the PE stream.  Only drops loads with empty sync_info.
    """
    import concourse.mybir as mybir

    for blk in nc.m.functions[0].blocks:
        insts = list(blk.instructions)
        keep = []
        remap = {}
        last_sig = None
        for i in insts:
            if i.engine == mybir.EngineType.PE:
                tn = type(i).__name__
                if tn == "InstLdweights":
                    ap = i.ins[0]
                    sig = (ap.memref, ap.offset, str(ap.ap), str(ap.dtype),
                           bool(i.is_transpose), str(i.perf_mode))
                    si = i.sync_info
                    clean = si is None or (
                        not si.on_wait and not si.on_update
                    )
                    if sig == last_sig and clean:
                        remap[i.name] = last_name
                        continue
                    last_sig = sig
                    last_name = i.name
                elif tn != "InstMatmult":
                    last_sig = None
            keep.append(i)
        if remap:
            for i in keep:
                i.remap_dependency_names(remap)
            blk.instructions = keep


def build_program(h: int = H, w: int = W):
    """Build the per-core Bass program (SPMD, identical on all cores)."""
    import concourse.mybir as mybir
    import concourse.tile as tile
    from concourse import bacc

    f32 = mybir.dt.float32
    bf16 = mybir.dt.bfloat16
    fp8 = mybir.dt.float8e4
    u8 = mybir.dt.uint8

    nt_h = h // P
    nt_w = w // P
    groups = GROUPS if h == H else [(a, min(b, h)) for a, b in GROUPS if a < h]
    n_q = max(1, w // BANK)
    q_w = w // n_q

    nc = bacc.Bacc("TRN2", target_bir_lowering=False, debug=False)
    # x is the raw f32 image reinterpreted as bf16 [h, 2w]: the odd
    # columns are the f32 values' top halves, exactly bf16 {0.0, 1.0}.
    x_ap = nc.dram_tensor("x", [h, 2 * w], bf16, kind="ExternalInput").ap()
    band8_ap = nc.dram_tensor("band8", [P, BANDW], fp8, kind="ExternalInput").ap()
    aux_ap = nc.dram_tensor("aux16", [P, 320], bf16, kind="ExternalInput").ap()
    y_ap = nc.dram_tensor("y", [h, w], u8, kind="ExternalOutput").ap()

    group_edges = sorted({a for a, _ in groups} | {b for _, b in groups})
    pieces_h = _pieces_for_pass(h, cuts=group_edges)
    pieces_w = _pieces_for_pass(w, cuts=range(BANK, w, BANK))
    ph_by_group = {
        gi: _flag_pieces(
            [p for p in pieces_h if glo <= p[1] < ghi], tile_base=glo
        )
        for gi, (glo, ghi) in enumerate(groups)
    }
    pw_by_q = {
        qf: _flag_pieces(
            [p for p in pieces_w if qf * q_w <= p[1] < (qf + 1) * q_w],
            tile_base=qf * q_w,
        )
        for qf in range(n_q)
    }

    OGRP = 2  # output row-strips per store DMA
    n_ogrp = max(1, nt_h // OGRP)
    ogrp = nt_h // n_ogrp

    with tile.TileContext(nc) as tc, ExitStack() as ctx:
        band_pool = ctx.enter_context(tc.tile_pool(name="band", bufs=1))
        xf_pool = ctx.enter_context(tc.tile_pool(name="xf", bufs=nt_h))
        s1_pool = ctx.enter_context(tc.tile_pool(name="s1", bufs=1))
        psA_pool = ctx.enter_context(tc.tile_pool(name="psA", bufs=4, space="PSUM"))
        psF_pool = ctx.enter_context(tc.tile_pool(name="psF", bufs=1, space="PSUM"))
        psB_pool = ctx.enter_context(tc.tile_pool(name="psB", bufs=3, space="PSUM"))
        out_pool = ctx.enter_context(tc.tile_pool(name="out", bufs=4))

        band8_t = band_pool.tile([P, BANDW], fp8, tag="band8")
        nc.sync.dma_start(out=band8_t[:], in_=band8_ap[:, :])
        aux_t = band_pool.tile([P, 320], bf16, tag="aux16")
        nc.sync.dma_start(out=aux_t[:], in_=aux_ap[:, :])
        bandn_t = aux_t[:, :BANDW]
        ones_stat = aux_t[0:16, A_ONES : A_ONES + P]
        top_mov = aux_t[0:16, A_TOP : A_TOP + 16]
        bot_mov = aux_t[0:16, A_BOT : A_BOT + 16]

        # Persistent PSUM tiles (same memref -> PE WAW stays program-order).
        # psA pair-tiles hold two adjacent w-strips' pass-1 windows side by
        # side (one 512-col bank each).
        psA_tiles = [
            psA_pool.tile([P, BANK], f32, tag="psA", name=f"psA{i}")
            for i in range(4)
        ]
        psB_tiles = [
            psB_pool.tile([P, BANK], f32, tag="psB", name=f"psB{i}")
            for i in range(3)
        ]
        psF = psF_pool.tile([P, BANK], f32, tag="psF")
        nB = [0]

        def next_psB():
            t = psB_tiles[nB[0] % len(psB_tiles)]
            nB[0] += 1
            return t

        # PE warm-up: throwaway matmuls on the band tile while the first
        # input strips stream in.  Sustained PE activity lifts the HAM
        # clock gate (1.2 -> 2.4 GHz) before the real work arrives.  psB0
        # is clobbered; its first real matmul starts with start=True.
        for _ in range(64):
            nc.tensor.matmul(
                psB_tiles[0][:, :64],
                band8_t[:, :P],
                band8_t[:, :64],
                start=True,
                stop=True,
            )

        # Input: contiguous row strips of the raw f32 bytes as bf16
        # [128, 2w]; consumed directly by pass-1 LDWEIGHTS at stride 2.
        # All strips stay resident, so this stream never blocks.
        xf_by_kt = {}
        for kt in range(nt_h):
            xf = xf_pool.tile([P, 2 * w], bf16, tag="xf")
            nc.sync.dma_start(
                out=xf[:], in_=x_ap[kt * P : (kt + 1) * P, :]
            )
            xf_by_kt[kt] = xf

        # S1^T strips: one [128, nt_w*h] fp8 tensor (counts 0..33, fp8
        # rounding preserves zero vs nonzero), column block wt holds
        # S1^T[wt] = [w', h].
        s1_t = s1_pool.tile([P, nt_w * h], fp8, tag="s1")
        s13 = s1_t[:].rearrange("p (wt ht) -> p wt ht", wt=nt_w)

        yt_tiles = {}
        done_ht = {}

        # Drains and evacs each alternate deterministically across DVE
        # and ACT (indexed, not flip-shared), so both engines carry half
        # of each stream and each queue's op order matches the order the
        # PE produces their PSUM inputs (no head-of-line blocking).
        # drain: psA (= n0 - 33) -> s1; DVE binarizes via is_gt -32.5,
        #   ACT recentres to raw counts via Copy bias 33 (mixing {0,1}
        #   and count blocks is fine: pass 2 needs zero vs nonzero).
        # evac: psB (>= 0) -> final {0,1} uint8.
        def drain_op(idx, dst_ap, src_ap):
            if idx % 2 == 0:
                nc.vector.tensor_scalar(
                    dst_ap, src_ap, -32.5, None, mybir.AluOpType.is_gt
                )
            else:
                nc.scalar.activation(
                    dst_ap, src_ap, mybir.ActivationFunctionType.Copy,
                    bias=33.0, scale=1.0,
                )

        def evac_bin(idx, dst_ap, src_ap):
            if idx % 2 == 0:
                nc.scalar.sign(dst_ap, src_ap)
            else:
                nc.vector.tensor_scalar(
                    dst_ap, src_ap, 0.5, None, mybir.AluOpType.is_gt
                )

        def p2_unit(ht, qf):
            """One pass-2 unit: fill a psB quarter for row-strip ht, evac
            it, and store the row-group when complete."""
            og, a = divmod(ht, ogrp)
            if og not in yt_tiles:
                yt_tiles[og] = out_pool.tile(
                    [P, ogrp * w], u8, tag="yt", name=f"yt{og}"
                )
                done_ht[og] = 0
            yt = yt_tiles[og]
            psB = next_psB()
            for wt, lo, hi, st, sp in pw_by_q[qf]:
                base = P * wt - RADIUS
                nc.tensor.matmul(
                    psB[:, lo - qf * q_w : hi - qf * q_w],
                    s1_t[:, wt * h + ht * P : wt * h + (ht + 1) * P],
                    band8_t[:, lo - base : hi - base],
                    start=st,
                    stop=sp,
                )
            if og == n_ogrp - 1:
                # Terminal chain: halve the evacuation latency by running
                # the two halves on DVE and ACT in parallel, and store the
                # final rows per strip so the very last store is 256KB.
                base = a * w + qf * q_w
                mid = q_w // 2
                nc.vector.tensor_scalar(
                    yt[:, base : base + mid], psB[:, :mid],
                    0.5, None, mybir.AluOpType.is_gt,
                )
                nc.scalar.sign(
                    yt[:, base + mid : base + q_w], psB[:, mid:q_w]
                )
                if qf == n_q - 1:
                    nc.sync.dma_start(
                        out=y_ap[ht * P : (ht + 1) * P, :],
                        in_=yt[:, a * w : (a + 1) * w],
                    )
                done_ht[og] += 1
                return
            evac_bin(
                ht * n_q + qf,
                yt[:, a * w + qf * q_w : a * w + (qf + 1) * q_w],
                psB[:, :q_w],
            )
            done_ht[og] += 1
            if done_ht[og] == ogrp * n_q:
                dst = y_ap[og * ogrp * P : (og + 1) * ogrp * P, :].rearrange(
                    "(a p) w -> p a w", p=P
                )
                nc.sync.dma_start(
                    out=dst, in_=yt[:].rearrange("p (a w) -> p a w", a=ogrp)
                )

        # pending pass-2 units from the previous group, interleaved into
        # this group's pass 1 so the PE always has independent work while
        # DVE/ACT drain the other pass's PSUM tiles.
        pending_p2 = []

        for gi, (glo, ghi) in enumerate(groups):
            gw = ghi - glo
            edge = "top" if gi == 0 else ("bot" if gi == len(groups) - 1
                                          else None)
            # Pass 1 for this group: psum cols are h_out in [glo, ghi);
            # w-strip pairs share one psA tile (bank 0 / bank 1).
            for wt in range(nt_w):
                psA = psA_tiles[wt % 4]
                side = 0
                if edge == "top":
                    nc.tensor.matmul(
                        psA[:, side : side + 16],
                        ones_stat, top_mov, start=True, stop=False,
                    )
                elif edge == "bot":
                    nc.tensor.matmul(
                        psA[:, side + gw - 16 : side + gw],
                        ones_stat, bot_mov, start=True, stop=False,
                    )
                for kt, lo, hi, st, sp in ph_by_group[gi]:
                    base = P * kt - RADIUS
                    xv = xf_by_kt[kt][:].rearrange(
                        "p (c two) -> p c two", two=2
                    )
                    nc.tensor.matmul(
                        psA[:, side + lo - glo : side + hi - glo],
                        xv[:, wt * P : (wt + 1) * P, 1:2],
                        bandn_t[:, lo - base : hi - base],
                        start=st and edge is None,
                        stop=sp,
                    )
                for _ in range(FILLERS.get(gi, 0)):
                    # Filler matmul on the dedicated bank: keeps the PE
                    # HAM clock warm through the DMA-paced early groups
                    # (an idle PE re-throttles to half clock).  No data
                    # deps, result discarded.
                    nc.tensor.matmul(
                        psF[:, :BANK], band8_t[:, :P],
                        xf_by_kt[0][:, :BANK], start=True, stop=True,
                    )
                if gi == len(groups) - 1:
                    # terminal chain: split each drain across DVE+ACT in
                    # parallel to halve its latency
                    mid = gw // 2
                    nc.vector.tensor_scalar(
                        s13[:, wt : wt + 1, glo : glo + mid],
                        psA[:, :mid],
                        -32.5, None, mybir.AluOpType.is_gt,
                    )
                    nc.scalar.activation(
                        s13[:, wt : wt + 1, glo + mid : ghi],
                        psA[:, mid:gw],
                        mybir.ActivationFunctionType.Copy,
                        bias=33.0, scale=1.0,
                    )
                else:
                    drain_op(wt, s13[:, wt : wt + 1, glo:ghi], psA[:, :gw])
                if pending_p2:
                    p2_unit(*pending_p2.pop(0))
            pending_p2.extend(
                (ht, qf)
                for ht in range(glo // P, ghi // P)
                for qf in range(n_q)
            )
            if gi < len(groups) - 1:
                # Flush this group's pass-2 eagerly, BEFORE the next
                # group's pass-1 enters the in-order PE queue: the next
                # group's matmuls wait on input strips, and ready pass-2
                # units emitted behind them would head-of-line block.
                while pending_p2:
                    p2_unit(*pending_p2.pop(0))

        for u in pending_p2:
            p2_unit(*u)

    _dedupe_ldweights(nc)
    nc.compile()
    return nc


def make_in_maps(imgs: np.ndarray):
    """Per-core input dicts from the [N, h, w] f32 image stack."""
    xbf = imgs.view(ml_dtypes.bfloat16)  # [N, h, 2w], same bytes
    band8 = band_np().astype(ml_dtypes.float8_e4m3)
    aux = aux_np()
    return [
        {"x": xbf[c], "band8": band8, "aux16": aux}
        for c in range(imgs.shape[0])
    ]


def kernel(x: np.ndarray) -> np.ndarray:
    """Full-input entry point: x (8, 2048, 2048, 1) f32 -> y same shape uint8."""
    from concourse.bass_utils import run_bass_kernel_spmd

    x = np.asarray(x)
    assert x.shape == (N_CORES, H, W, 1), x.shape
    imgs = np.ascontiguousarray(x[:, :, :, 0], dtype=np.float32)

    nc = build_program(H, W)
    res = run_bass_kernel_spmd(nc, make_in_maps(imgs), list(range(N_CORES)))
    y = np.stack([res.results[c]["y"] for c in range(N_CORES)])
    return y[..., None]
